# revision 35
# baseline (speedup 1.0000x reference)
"""Trainium2 Bass kernel for pre-LN multi-head self-attention.

Problem shapes (hardcoded): q (4, 2048, 1024) f32, attn_mask (2048, 2048) bool,
Wq/Wk/Wv (1024, 1024) f32, Wo (1024, 1024) f32, gamma/beta (1024,) f32.
N_HEAD=16, D_HEAD=64, pre-layernorm, softmax over the key axis.

Sharding: 8 cores = 4 batches x 2 head-groups (8 heads each). Each core
computes LN(q_b), its head-slice of the QKV projections, attention for its
8 heads, and a partial O-projection. The O-projection "all-reduce" over the
2 head-groups of a batch plus the qn residual add happens at host gather time.

Final design (trace-driven; baseline 591us -> 454us measured):
  - Phase B is ACT-bound: 2 exp ACTIVATEs [128,1024] per jc is the floor
    (~2.3us/jc). The jc loop is software-pipelined: PE issue order is
    QK(jc) then AV(jc-1); heads ping-pong so QK slots under the other
    head's exp; each head's mask TT issues right after its exp. The
    pipeline (and each block's den/normalize tail) carries ACROSS
    (hp, ih) blocks so neither the exp stream nor the PE queue drains
    at block boundaries.
  - All stationary matmul operands are 128 columns wide (hv per-head slice
    padded 65->128) so FWL keeps matmuls pipelined at ~215ns instead of
    the ~378ns isolated latency.
  - fp8 e4m3 DoubleRow for the Q/K/V projections (weights x32 host-side,
    qnT fp8; hq/hk/hv come out x32 and live in bf16 -> the x1024 score
    scale folds into the exp scale) and for the O-projection (vecT8 =
    16*vec via the ones-column trick at WSCALE/VSCALE, wo8 = 32*Wo,
    host divides partials by 512). Scores/probs stay bf16.
  - LN rstd via ACT Sqrt + DVE reciprocal_approx_fast (one table set);
    qn and the output partials are bf16; QKV psum->sbuf copies on
    ScalarE (idle in phase A).
"""

import numpy as np
import ml_dtypes
from contextlib import ExitStack

import concourse.bass as bass
import concourse.tile as tile
from concourse import bacc, mybir
from concourse.bass_utils import run_bass_kernel_spmd

F32 = mybir.dt.float32
BF16 = mybir.dt.bfloat16
FP8 = mybir.dt.float8e4
Alu = mybir.AluOpType
Act = mybir.ActivationFunctionType
DR = mybir.MatmulPerfMode.DoubleRow

BSZ, SEQ, DM = 4, 2048, 1024
NH, DH = 16, 64
HPC = 8              # heads per core
HD = HPC * DH        # 512 = per-core slice of the head dim
NCORES = 8
SCALE = 1.0 / (DH ** 0.5)
LN_EPS = 1e-5

NT = SEQ // 128      # 16 seq tiles of 128
NDC = DM // 128      # 8 d_model chunks of 128
NHC = HD // 128      # 4 per-core head-dim chunks of 128
HVW = 128            # hv cols per head: 64 v-dims + ones col + pad (FWL)

WSCALE = 32.0        # host-side weight scale into fp8 e4m3 (all of W{q,k,v,o})
VSCALE = 16.0        # vecT8 carries 16x the true vec (fp8 sweet spot)
SSCALE = SCALE / (WSCALE * WSCALE)   # exp scale: scores carry x32 q and x32 k
PSCALE = VSCALE * WSCALE             # host divides part_out by this


def _mha_tile(ctx, tc, dq, dmask, dwq, dwk, dwv, dwo, dident,
              dgamma, dbeta, dqn, dpart, parts="full"):
    nc = tc.nc
    do_qkv = parts in ("qkv", "attn", "full")
    do_attn = parts in ("attn", "full")
    do_o = parts == "full"

    persist = ctx.enter_context(tc.tile_pool(name="persist", bufs=1))
    ident_sb = persist.tile([128, 128], BF16)
    nc.sync.dma_start(out=ident_sb, in_=dident)
    eps_sb = persist.tile([128, 1], F32)
    nc.vector.memset(eps_sb, LN_EPS)

    hqT = persist.tile([128, NHC * SEQ], BF16)      # [hd-chunk part, seq], x32
    hkT = persist.tile([128, NHC * SEQ], BF16)      # x32
    hv = persist.tile([128, NT * HPC * HVW], BF16)  # [j part, per-jc 8*128], x32
    vecT = persist.tile([128, NHC * SEQ], BF16)     # unnormalized, [hd, i]
    vecT8 = persist.tile([128, NHC * SEQ], FP8)     # normalized, x16 scale
    wo_sb = persist.tile([128, NHC * DM], FP8)

    gamma_sb = beta_sb = None
    if dgamma is not None:
        gamma_sb = persist.tile([128, DM], F32)
        beta_sb = persist.tile([128, DM], F32)
        nc.sync.dma_start(out=gamma_sb, in_=bass.AP(
            tensor=dgamma.tensor, offset=dgamma.offset,
            ap=[[0, 128]] + list(dgamma.ap)))
        nc.sync.dma_start(out=beta_sb, in_=bass.AP(
            tensor=dbeta.tensor, offset=dbeta.offset,
            ap=[[0, 128]] + list(dbeta.ap)))

    # ---------------- Phase A: LN + transpose + QKV projections -------------
    with tc.tile_pool(name="phA", bufs=1) as pA, \
         tc.tile_pool(name="qtiles", bufs=2) as qpool, \
         tc.tile_pool(name="qnbf", bufs=2) as qnbfpool, \
         tc.tile_pool(name="stats", bufs=6) as spool, \
         tc.tile_pool(name="psT", bufs=2, space="PSUM") as psT, \
         tc.tile_pool(name="psQK", bufs=3, space="PSUM") as psQK:

        qnT = pA.tile([128, NDC * SEQ], FP8)        # [dm-chunk part, seq]
        qnT3 = qnT.rearrange("p (dc s) -> p dc s", s=SEQ)
        wq_sb = pA.tile([128, NDC * HD], FP8)
        wk_sb = pA.tile([128, NDC * HD], FP8)
        wv_sb = pA.tile([128, NDC * HD], FP8)
        wq3 = wq_sb.rearrange("p (dc hd) -> p dc hd", hd=HD)
        wk3 = wk_sb.rearrange("p (dc hd) -> p dc hd", hd=HD)
        wv3 = wv_sb.rearrange("p (dc hd) -> p dc hd", hd=HD)

        for tb in range(NT // 4):
            # one merged DMA loads 4 seq tiles of q
            rows4 = slice(tb * 512, (tb + 1) * 512)
            qt = qpool.tile([128, 4, DM], F32, tag="qt")
            q4 = dq[rows4, :].rearrange("(a p) m -> p a m", p=128)
            if tb == 0:
                # split the first load so LN(k=0) starts ~4us earlier
                nc.sync.dma_start(out=qt[:, 0:2, :], in_=q4[:, 0:2, :])
                nc.sync.dma_start(out=qt[:, 2:4, :], in_=q4[:, 2:4, :])
            else:
                nc.sync.dma_start(out=qt, in_=q4)
            if tb == 0:
                for w_sb, dw in ((wq_sb, dwq), (wk_sb, dwk), (wv_sb, dwv)):
                    nc.sync.dma_start(
                        out=w_sb.rearrange("p (dc hd) -> p dc hd", hd=HD),
                        in_=dw.rearrange("(dc p) hd -> p dc hd", p=128))
            qnbf = qnbfpool.tile([128, 4, DM], BF16, tag="qnbf")
            for k in range(4):
                st = spool.tile([128, 2, 6], F32, tag="st")
                nc.vector.bn_stats(out=st[:, 0, :], in_=qt[:, k, 0:512])
                nc.vector.bn_stats(out=st[:, 1, :], in_=qt[:, k, 512:1024])
                mv = spool.tile([128, 2], F32, tag="mv")
                nc.vector.bn_aggr(out=mv, in_=st)
                # rstd = 1/sqrt(var+eps): Sqrt on ACT (one table set for all
                # of phase A), reciprocal on DVE (~18-bit, plenty).
                std = spool.tile([128, 1], F32, tag="std")
                nc.scalar.activation(out=std, in_=mv[:, 1:2], func=Act.Sqrt,
                                     bias=eps_sb, scale=1.0)
                rstd = spool.tile([128, 1], F32, tag="rstd")
                nc.vector.reciprocal_approx_fast(out=rstd, in_=std)
                negmr = spool.tile([128, 1], F32, tag="negmr")
                nc.vector.tensor_tensor(out=negmr, in0=mv[:, 0:1], in1=rstd,
                                        op=Alu.mult)
                nc.vector.tensor_scalar_mul(negmr, negmr, -1.0)
                if gamma_sb is None:
                    nc.vector.tensor_scalar(out=qnbf[:, k, :], in0=qt[:, k, :],
                                            scalar1=rstd, scalar2=negmr,
                                            op0=Alu.mult, op1=Alu.add)
                else:
                    qnf = qt
                    nc.vector.tensor_scalar(out=qnf[:, k, :], in0=qt[:, k, :],
                                            scalar1=rstd, scalar2=negmr,
                                            op0=Alu.mult, op1=Alu.add)
                    nc.vector.tensor_tensor(out=qnf[:, k, :], in0=qnf[:, k, :],
                                            in1=gamma_sb, op=Alu.mult)
                    nc.vector.tensor_tensor(out=qnbf[:, k, :], in0=qnf[:, k, :],
                                            in1=beta_sb, op=Alu.add)
            nc.gpsimd.dma_start(
                out=dqn[rows4, :].rearrange("(a p) m -> p a m", p=128), in_=qnbf)
            for dc in range(NDC):
                pst = psT.tile([128, 512], BF16, tag="pst")
                for k in range(4):
                    nc.tensor.transpose(pst[:, k * 128:(k + 1) * 128],
                                        qnbf[:, k, dc * 128:(dc + 1) * 128],
                                        ident_sb)
                nc.scalar.copy(
                    out=qnT[:, dc * SEQ + tb * 512: dc * SEQ + (tb + 1) * 512],
                    in_=pst)

            # QKV for this seq block (sc == tb), fp8 DoubleRow over dc pairs:
            # overlaps the next block's LN. hq/hk/hv come out x32 (weights
            # are scaled into e4m3 host-side) and are stored bf16.
            if not do_qkv:
                continue
            sc = tb
            ssl = slice(sc * 512, (sc + 1) * 512)
            for w3, dstT in ((wq3, hqT), (wk3, hkT)):
                for hc in range(NHC):
                    ps = psQK.tile([128, 512], F32, tag="psqk")
                    for dcp in range(NDC // 2):
                        nc.tensor.matmul(
                            ps,
                            lhsT=w3[:, 2 * dcp:2 * dcp + 2, hc * 128:(hc + 1) * 128],
                            rhs=qnT3[:, 2 * dcp:2 * dcp + 2, ssl],
                            start=(dcp == 0), stop=(dcp == NDC // 2 - 1),
                            perf_mode=DR)
                    nc.scalar.copy(
                        out=dstT[:, hc * SEQ + sc * 512: hc * SEQ + (sc + 1) * 512],
                        in_=ps)
            for jc in range(4 * tb, 4 * tb + 4):
                ps = psQK.tile([128, HD], F32, tag="psv")
                jsl = slice(jc * 128, (jc + 1) * 128)
                for dcp in range(NDC // 2):
                    nc.tensor.matmul(
                        ps,
                        lhsT=qnT3[:, 2 * dcp:2 * dcp + 2, jsl],
                        rhs=wv3[:, 2 * dcp:2 * dcp + 2, :],
                        start=(dcp == 0), stop=(dcp == NDC // 2 - 1),
                        perf_mode=DR)
                blk = hv[:, jc * HPC * HVW:(jc + 1) * HPC * HVW]
                blk3 = blk.rearrange("p (h x) -> p h x", x=HVW)
                nc.scalar.copy(out=blk3[:, :, 0:DH],
                               in_=ps.rearrange("p (h x) -> p h x", x=DH))
        # keep the PE busy across the A->B boundary: scattered idle here
        # trips the HAM MID window and the throttle then sticks at K=4/8
        # for ~75us of phase B. These writes go to a dead psQK bank.
        for _w in range(24):
            psd = psQK.tile([128, 512], F32, tag="psqk")
            nc.tensor.matmul(psd, lhsT=ident_sb, rhs=hqT[:, 0:512],
                             start=True, stop=True)
        hv4 = hv.rearrange("p (j h x) -> p j h x", h=HPC, x=HVW)
        # hv carries x32 values; ones column = WSCALE/VSCALE so den =
        # (WSCALE/VSCALE)*sum(p) and the recip-normalized vecT8 =
        # VSCALE * true vec (fp8 e4m3 sweet spot)
        nc.vector.memset(hv4[:, :, :, DH:DH + 1], WSCALE / VSCALE)
        nc.vector.memset(hv4[:, :, :, DH + 1:HVW], 0.0)

    # ---------------- Phase B: attention, ACT-bound ping-pong ---------------
    drecip = nc.dram_tensor(f"recip_scratch{nc.next_id()}", [HPC, SEQ], F32).ap()
    with tc.tile_pool(name="mk", bufs=1) as mkpool, \
         tc.tile_pool(name="pp", bufs=6) as ppool, \
         tc.tile_pool(name="stg", bufs=2) as stpool, \
         tc.tile_pool(name="den", bufs=1) as denpool, \
         tc.tile_pool(name="sps", bufs=1, space="PSUM") as spsum, \
         tc.tile_pool(name="vps", bufs=1, space="PSUM") as vpsum:
        # whole mask resident [j-part, jc, i]; 16 chunk DMAs stream in jc order
        mask_all = mkpool.tile([128, NT, SEQ], BF16)
        for c in range(NT):
            nc.gpsimd.dma_start(out=mask_all[:, c, :],
                                in_=dmask[c * 128:(c + 1) * 128, :])
        nc.sync.dma_start(
            out=wo_sb.rearrange("p (hc m) -> p hc m", m=DM),
            in_=dwo.rearrange("(hc p) m -> p hc m", p=128))
        if not do_attn:
            nc.vector.memset(vecT8, 0.0)

        def issue_av(pab, jc, vA, vB, ha, hb):
            va_l = hv[:, jc * HPC * HVW + ha * HVW: jc * HPC * HVW + (ha + 1) * HVW]
            vb_l = hv[:, jc * HPC * HVW + hb * HVW: jc * HPC * HVW + (hb + 1) * HVW]
            for n in range(2):
                osl = slice(n * 512, (n + 1) * 512)
                nc.tensor.matmul(vA[:, osl], lhsT=va_l, rhs=pab[:, n * 512:(n + 1) * 512],
                                 start=(jc == 0), stop=(jc == NT - 1))
                nc.tensor.matmul(vB[:, osl], lhsT=vb_l,
                                 rhs=pab[:, 1024 + n * 512: 1024 + (n + 1) * 512],
                                 start=(jc == 0), stop=(jc == NT - 1))

        def issue_tail(vA, vB, ha, hb, hp, ih):
            # den/vec extraction + normalize for a finished block. Must be
            # issued AFTER that block's last AV (tracker orders by issue).
            # Two psum reads total: rows 0:65 of each head (row 64 = den)
            # ride along with the vec rows; den goes on from SBUF by DMA.
            isl = slice(hp * SEQ + ih * 1024, hp * SEQ + (ih + 1) * 1024)
            dsl = slice(ih * 1024, (ih + 1) * 1024)
            nc.vector.tensor_copy(out=vecT[0:65, isl], in_=vA[0:65, :])
            stage = stpool.tile([65, 1024], BF16, tag="stg")
            nc.vector.tensor_copy(out=stage, in_=vB[0:65, :])
            den_ih = denpool.tile([2, 1024], BF16, tag="den")
            nc.sync.dma_start(out=den_ih[0:1, :], in_=vecT[64:65, isl])
            nc.sync.dma_start(out=den_ih[1:2, :], in_=stage[64:65, :])
            # head-b vec rows move from sbuf 0:64 to 64:128 (cross-partition,
            # DMA only); this also overwrites the temporary den_a row.
            nc.gpsimd.dma_start(out=vecT[64:128, isl], in_=stage[0:64, :])
            # normalize this chunk (into fp8 vecT8, x16 scale) while the
            # next chunk's attention runs
            denf = denpool.tile([2, 1024], F32, tag="denf")
            nc.vector.tensor_copy(out=denf, in_=den_ih)
            recip_ih = denpool.tile([2, 1024], F32, tag="recip")
            nc.vector.reciprocal_approx_fast(out=recip_ih, in_=denf)
            nc.sync.dma_start(out=drecip[ha:hb + 1, dsl], in_=recip_ih)
            bc_ih = denpool.tile([128, 1024], F32, tag="bc")
            for h, lo in ((ha, 0), (hb, 64)):
                row = drecip[h:h + 1, dsl]
                nc.sync.dma_start(
                    out=bc_ih[lo:lo + 64, :],
                    in_=bass.AP(tensor=row.tensor, offset=row.offset,
                                ap=[[0, 64]] + list(row.ap[1:])))
            nc.vector.tensor_tensor(out=vecT8[:, isl], in0=vecT[:, isl],
                                    in1=bc_ih, op=Alu.mult)

        # the AV-delay pipeline (depth 2) carries ACROSS (hp, ih) blocks: a
        # block's last AVs and its den/normalize tail issue behind the next
        # block's first QKs, so neither the ACT exp stream nor the PE queue
        # drains at block boundaries (the refill was costing ~5.5us x 16
        # tails; depth 2 also gives the tail's DVE copies two periods to
        # clear the vA/vB banks before the next block's first AV needs them).
        prevs = []
        pending_tail = None
        for hp in range(HPC // 2 if do_attn else 0):
            ha, hb = 2 * hp, 2 * hp + 1
            for ih in range(2):
                # [128, 1024]: rows 0:64 vec, row 64 denominator, 65:128 pad
                vA = vpsum.tile([128, 1024], F32, tag="vA")
                vB = vpsum.tile([128, 1024], F32, tag="vB")
                for jc in range(NT):
                    sA = spsum.tile([128, 1024], F32, tag="sA")
                    sB = spsum.tile([128, 1024], F32, tag="sB")
                    kslice = slice(hp * SEQ + jc * 128, hp * SEQ + (jc + 1) * 128)
                    for n in range(2):
                        qsl = slice(hp * SEQ + ih * 1024 + n * 512,
                                    hp * SEQ + ih * 1024 + (n + 1) * 512)
                        osl = slice(n * 512, (n + 1) * 512)
                        nc.tensor.matmul(sA[:, osl], lhsT=hkT[0:64, kslice],
                                         rhs=hqT[0:64, qsl], start=True, stop=True)
                        nc.tensor.matmul(sB[:, osl], lhsT=hkT[64:128, kslice],
                                         rhs=hqT[64:128, qsl], start=True, stop=True)
                    # PE pipeline: AV trails QK by one jc iteration; it becomes
                    # ready (mask done) while exp(jc) occupies ACT.
                    if len(prevs) == 1:
                        done = prevs.pop(0)
                        issue_av(*done)
                        if done[1] == NT - 1 and pending_tail is not None:
                            issue_tail(*pending_tail)
                            pending_tail = None
                    # per-head exp + mask: masking head a right after its exp
                    # (in parallel with head b's exp) shortens the
                    # exp -> mask -> AV -> QK -> exp dependency chain
                    pab = ppool.tile([128, 2048], BF16, tag="pab")
                    mk = mask_all[:, jc, ih * 1024:(ih + 1) * 1024]
                    nc.scalar.activation(out=pab[:, 0:1024], in_=sA,
                                         func=Act.Exp, scale=SSCALE)
                    nc.vector.tensor_tensor(out=pab[:, 0:1024],
                                            in0=pab[:, 0:1024], in1=mk,
                                            op=Alu.mult)
                    nc.scalar.activation(out=pab[:, 1024:2048], in_=sB,
                                         func=Act.Exp, scale=SSCALE)
                    nc.vector.tensor_tensor(out=pab[:, 1024:2048],
                                            in0=pab[:, 1024:2048], in1=mk,
                                            op=Alu.mult)
                    prevs.append((pab, jc, vA, vB, ha, hb))
                pending_tail = (vA, vB, ha, hb, hp, ih)
        for done in prevs:
            issue_av(*done)
            if done[1] == NT - 1 and pending_tail is not None:
                issue_tail(*pending_tail)
                pending_tail = None

    # ---------------- Phase C: O-projection (fp8 DoubleRow) -----------------
    vec3 = vecT8.rearrange("p (hc s) -> p hc s", s=SEQ)
    wo3 = wo_sb.rearrange("p (hc m) -> p hc m", m=DM)
    with tc.tile_pool(name="po", bufs=4, space="PSUM") as opool, \
         tc.tile_pool(name="outs", bufs=2) as outpool:
        for ob in range(NT // 4):
            outt = outpool.tile([128, 4, DM], BF16, tag="outt")
            if not do_o:
                nc.vector.memset(outt, 0.0)
                nc.sync.dma_start(
                    out=dpart[ob * 512:(ob + 1) * 512, :]
                        .rearrange("(a p) m -> p a m", p=128),
                    in_=outt)
                continue
            for k in range(4):
                it = ob * 4 + k
                isl = slice(it * 128, (it + 1) * 128)
                for mc in range(2):
                    po = opool.tile([128, 512], F32, tag="po")
                    for hcp in range(NHC // 2):
                        nc.tensor.matmul(
                            po,
                            lhsT=vec3[:, 2 * hcp:2 * hcp + 2, isl],
                            rhs=wo3[:, 2 * hcp:2 * hcp + 2, mc * 512:(mc + 1) * 512],
                            start=(hcp == 0), stop=(hcp == NHC // 2 - 1),
                            perf_mode=DR)
                    nc.vector.tensor_copy(out=outt[:, k, mc * 512:(mc + 1) * 512],
                                          in_=po)
            nc.sync.dma_start(
                out=dpart[ob * 512:(ob + 1) * 512, :]
                    .rearrange("(a p) m -> p a m", p=128),
                in_=outt)


_NC_CACHE = {}


def _build(gamma_trivial, repeat=1, parts="full"):
    key = (bool(gamma_trivial), repeat, parts)
    if key in _NC_CACHE:
        return _NC_CACHE[key]
    nc = bacc.Bacc("TRN2", target_bir_lowering=False, debug=False,
                   num_devices=NCORES)
    dq = nc.dram_tensor("q", [SEQ, DM], F32, kind="ExternalInput").ap()
    dmask = nc.dram_tensor("maskt", [SEQ, SEQ], BF16, kind="ExternalInput").ap()
    dwq = nc.dram_tensor("wq", [DM, HD], FP8, kind="ExternalInput").ap()
    dwk = nc.dram_tensor("wk", [DM, HD], FP8, kind="ExternalInput").ap()
    dwv = nc.dram_tensor("wv", [DM, HD], FP8, kind="ExternalInput").ap()
    dwo = nc.dram_tensor("wo", [HD, DM], FP8, kind="ExternalInput").ap()
    dident = nc.dram_tensor("ident", [128, 128], BF16, kind="ExternalInput").ap()
    dgamma = dbeta = None
    if not gamma_trivial:
        dgamma = nc.dram_tensor("gamma", [DM], F32, kind="ExternalInput").ap()
        dbeta = nc.dram_tensor("beta", [DM], F32, kind="ExternalInput").ap()
    dqn = nc.dram_tensor("qn_out", [SEQ, DM], BF16, kind="ExternalOutput").ap()
    dpart = nc.dram_tensor("part_out", [SEQ, DM], BF16, kind="ExternalOutput").ap()
    with tile.TileContext(nc) as tc:
        for _rep in range(repeat):
            with ExitStack() as ctx:
                _mha_tile(ctx, tc, dq, dmask, dwq, dwk, dwv, dwo, dident,
                          dgamma, dbeta, dqn, dpart, parts=parts)
    nc.compile()
    _NC_CACHE[key] = nc
    return nc


def _run(nc, in_maps, **kwargs):
    return run_bass_kernel_spmd(nc, in_maps, list(range(NCORES)), **kwargs)


def make_in_maps(q, attn_mask, Wq, Wk, Wv, Wo, gamma, beta, gamma_trivial):
    bf = ml_dtypes.bfloat16
    f8 = ml_dtypes.float8_e4m3
    q = np.ascontiguousarray(np.asarray(q, dtype=np.float32))
    maskt = np.ascontiguousarray(
        (~np.asarray(attn_mask, dtype=bool)).T.astype(bf))
    Wq = np.asarray(Wq, dtype=np.float32)
    Wk = np.asarray(Wk, dtype=np.float32)
    Wv = np.asarray(Wv, dtype=np.float32)
    Wo = np.asarray(Wo, dtype=np.float32)
    ident = np.eye(128, dtype=bf)
    in_maps = []
    for c in range(NCORES):
        b, g = c // 2, c % 2
        cols = slice(g * HD, (g + 1) * HD)
        def to8(a):
            return np.ascontiguousarray(
                np.clip(a * WSCALE, -240.0, 240.0).astype(f8))

        m = {
            "q": q[b],
            "maskt": maskt,
            "wq": to8(Wq[:, cols]),
            "wk": to8(Wk[:, cols]),
            "wv": to8(Wv[:, cols]),
            "wo": to8(Wo[cols, :]),
            "ident": ident,
        }
        if not gamma_trivial:
            m["gamma"] = np.asarray(gamma, dtype=np.float32)
            m["beta"] = np.asarray(beta, dtype=np.float32)
        in_maps.append(m)
    return in_maps


def kernel(q, attn_mask, Wq, Wk, Wv, Wo, gamma, beta):
    gamma_np = np.asarray(gamma, dtype=np.float32)
    beta_np = np.asarray(beta, dtype=np.float32)
    gamma_trivial = bool(np.all(gamma_np == 1.0) and np.all(beta_np == 0.0))
    nc = _build(gamma_trivial)
    in_maps = make_in_maps(q, attn_mask, Wq, Wk, Wv, Wo, gamma_np, beta_np,
                           gamma_trivial)
    res = _run(nc, in_maps).results
    out = np.empty((BSZ, SEQ, DM), dtype=np.float32)
    inv = 1.0 / PSCALE
    for b in range(BSZ):
        out[b] = res[2 * b]["qn_out"].astype(np.float32)
        out[b] += res[2 * b]["part_out"].astype(np.float32) * inv
        out[b] += res[2 * b + 1]["part_out"].astype(np.float32) * inv
    return out


if __name__ == "__main__":
    rng = np.random.default_rng(0)
    ins = {
        "q": rng.standard_normal((BSZ, SEQ, DM), dtype=np.float32),
        "attn_mask": rng.integers(0, 2, (SEQ, SEQ)).astype(bool),
        "Wq": rng.standard_normal((DM, NH * DH), dtype=np.float32) * 0.03,
        "Wk": rng.standard_normal((DM, NH * DH), dtype=np.float32) * 0.03,
        "Wv": rng.standard_normal((DM, NH * DH), dtype=np.float32) * 0.03,
        "Wo": rng.standard_normal((NH * DH, DM), dtype=np.float32) * 0.03,
        "gamma": np.ones(DM, np.float32),
        "beta": np.zeros(DM, np.float32),
    }
    out = kernel(**ins)
    print("kernel ran, out shape", out.shape, out.dtype)
